# revision 5
# baseline (speedup 1.0000x reference)
"""HGNN conv kernel for Trainium2, 8 NeuronCores.

out = dv ⊙ (H @ (W·de ⊙ (H^T @ (dv ⊙ (x@weight))))) + bias
  dv = rowsum(H)^-1/2  [N], de = colsum(H)^-1  [E]
  N=16384, E=8192, F=64.

Sharding: H/x row-sharded over N across 8 cores (2048 rows each).
Host preps per-core bf16 H shard in both layouts (natural + transposed) —
a pure layout/precision transform; all FLOPs (matmuls, reductions,
scalings) run on device.

Device per core:
  pass 1: stream H natural [128,8192] row-tiles; DVE row-sums -> dv;
          xw = x@weight (PE); xs' = [dv*xw | 1] bf16 stationary;
          y^T[65,512-blk] += xs'^T @ H  (PSUM accum over 4-tile n-groups,
          DVE flush to f32 SBUF acc). Ones column yields colsum partials.
  AllReduce [65,8192] f32 across 8 cores.
  y2 = (W*de) * y_sum  via PE transpose + ACT scaled copy -> bf16 [e,64].
  pass 2: stream H^T [128e,512n] tiles; out^T[64,512] += y2^T @ H^T;
          PE transpose back, ACT copy scaled by dv, DVE bias add, DMA out.
"""

import numpy as np
import ml_dtypes

N, E, F = 16384, 8192, 64
NCORES = 8
NL = N // NCORES          # 2048 rows per core
P = 128
NT = NL // P              # 16 n-tiles per core
ET = E // P               # 64 e-tiles
EBLK = 512
EB = E // EBLK            # 16 e-blocks (pass 1 moving free dim)
NBLK = 512
NB = NL // NBLK           # 4 n-blocks (pass 2 moving free dim)
G = 4                     # n-tiles per PSUM accumulation group (pass 1)

_prog_cache = {}


def _build_program():
    import concourse.bass as bass
    import concourse.mybir as mybir
    import concourse.tile as tile
    from concourse import bacc
    from concourse.masks import make_identity

    f32 = mybir.dt.float32
    bf16 = mybir.dt.bfloat16
    Copy = mybir.ActivationFunctionType.Copy
    add = mybir.AluOpType.add
    mult = mybir.AluOpType.mult
    X = mybir.AxisListType.X

    nc = bacc.Bacc(
        "TRN2", target_bir_lowering=False, debug=False, num_devices=NCORES
    )
    h = nc.declare_dram_parameter("h", [NL, E], bf16, isOutput=False)
    ht = nc.declare_dram_parameter("ht", [E, NL], bf16, isOutput=False)
    xt = nc.declare_dram_parameter("xt", [F, NL], f32, isOutput=False)
    wmat = nc.declare_dram_parameter("wmat", [F, F], f32, isOutput=False)
    wstr = nc.declare_dram_parameter("wstr", [P, ET], f32, isOutput=False)
    biasb = nc.declare_dram_parameter("biasb", [P, F], f32, isOutput=False)
    out = nc.declare_dram_parameter("out", [NL, F], f32, isOutput=True)

    with tile.TileContext(nc) as tc:
        with (
            tc.tile_pool(name="hp", bufs=G + 2) as hp,           # H row tiles
            tc.tile_pool(name="xsp", bufs=G + 1) as xsp,         # xs' tiles
            tc.tile_pool(name="accp", bufs=1) as accp,           # y acc
            tc.tile_pool(name="smallp", bufs=1) as smallp,       # persistent small
            tc.tile_pool(name="rp", bufs=4) as rp,               # rowsum temps
            tc.tile_pool(name="htp", bufs=16) as htp,            # HT tiles
            tc.tile_pool(name="outp", bufs=4) as outp,           # out staging
            tc.tile_pool(name="ps_small", bufs=2, space="PSUM") as ps_small,
            tc.tile_pool(name="ps_big", bufs=3, space="PSUM") as ps_big,
            tc.tile_pool(name="dramp", bufs=1, space="DRAM") as dramp,
        ):
            # ---- persistent small tensors ----
            xt_sb = smallp.tile([F, NL], f32, tag="xt")
            nc.sync.dma_start(xt_sb[:], xt[:, :])
            wmat_sb = smallp.tile([F, F], f32, tag="wmat")
            nc.sync.dma_start(wmat_sb[:], wmat[:, :])
            wstr_sb = smallp.tile([P, ET], f32, tag="wstr")
            nc.sync.dma_start(wstr_sb[:], wstr[:, :])
            bias_sb = smallp.tile([P, F], f32, tag="bias")
            nc.sync.dma_start(bias_sb[:], biasb[:, :])
            ident = smallp.tile([F, F], f32, tag="ident")
            make_identity(nc, ident)
            dv_all = smallp.tile([P, NT], f32, tag="dv")
            y_acc = accp.tile([F + 1, E], f32, tag="yacc")

            # ---- pass 1: y^T[f,e] (+ colsum row) over n-groups ----
            for g in range(NT // G):
                group = []
                for i in range(G):
                    t = g * G + i
                    h_t = hp.tile([P, E], bf16, tag="h")
                    nc.sync.dma_start(h_t[:], h[t * P:(t + 1) * P, :])
                    # rowsum -> dv = sqrt(1/rowsum)
                    rsum = rp.tile([P, 1], f32, tag="rsum")
                    nc.vector.tensor_reduce(out=rsum[:], in_=h_t[:], axis=X, op=add)
                    rinv = rp.tile([P, 1], f32, tag="rinv")
                    nc.vector.reciprocal(out=rinv[:], in_=rsum[:])
                    nc.scalar.sqrt(out=dv_all[:, t:t + 1], in_=rinv[:])
                    # xw = x @ weight for this tile
                    xw_ps = ps_small.tile([P, F], f32, tag="xw")
                    nc.tensor.matmul(
                        xw_ps[:], lhsT=xt_sb[:, t * P:(t + 1) * P], rhs=wmat_sb[:],
                        start=True, stop=True,
                    )
                    xs_t = xsp.tile([P, F + 1], bf16, tag="xs")
                    nc.scalar.activation(
                        out=xs_t[:, 0:F], in_=xw_ps[:], func=Copy,
                        scale=dv_all[:, t:t + 1],
                    )
                    nc.gpsimd.memset(xs_t[:, F:F + 1], 1.0)
                    group.append((xs_t, h_t))
                for b in range(EB):
                    yps = ps_big.tile([F + 1, EBLK], f32, tag="yps")
                    for i, (xs_t, h_t) in enumerate(group):
                        nc.tensor.matmul(
                            yps[:], lhsT=xs_t[:], rhs=h_t[:, b * EBLK:(b + 1) * EBLK],
                            start=(i == 0), stop=(i == G - 1),
                        )
                    dst = y_acc[:, b * EBLK:(b + 1) * EBLK]
                    if g == 0:
                        nc.vector.tensor_copy(out=dst, in_=yps[:])
                    else:
                        nc.vector.tensor_tensor(out=dst, in0=dst, in1=yps[:], op=add)

            # ---- AllReduce partial y^T (+colsum row) across 8 cores ----
            bounce_in = dramp.tile([F + 1, E], f32)
            bounce_out = dramp.tile([F + 1, E], f32)
            nc.sync.dma_start(bounce_in[:], y_acc[:])
            nc.gpsimd.collective_compute(
                "AllReduce",
                mybir.AluOpType.add,
                ins=[bounce_in[:].opt()],
                outs=[bounce_out[:].opt()],
                replica_groups=[list(range(NCORES))],
            )
            nc.sync.dma_start(y_acc[:], bounce_out[:])

            # ---- y2 = (W * de) * y_sum, transposed to [e,64] bf16 ----
            cs = smallp.tile([P, ET], f32, tag="cs")
            nc.sync.dma_start(cs[:], bounce_out[F, :].rearrange("(o p) -> p o", p=P))
            de_t = smallp.tile([P, ET], f32, tag="de")
            nc.vector.reciprocal(out=de_t[:], in_=cs[:])
            wde = smallp.tile([P, ET], f32, tag="wde")
            nc.vector.tensor_tensor(out=wde[:], in0=de_t[:], in1=wstr_sb[:], op=mult)
            y2_sb = smallp.tile([P, ET, F], bf16, tag="y2")
            for t in range(ET):
                tp = ps_small.tile([P, F], f32, tag="tp")
                nc.tensor.transpose(tp[:], y_acc[0:F, t * P:(t + 1) * P], ident[:])
                nc.scalar.activation(
                    out=y2_sb[:, t, :], in_=tp[:], func=Copy, scale=wde[:, t:t + 1],
                )

            # ---- pass 2: out^T[64, nblk] = y2^T @ H^T, then untranspose ----
            for j in range(NB):
                ops = ps_big.tile([F, NBLK], f32, tag="yps")
                for t in range(ET):
                    htt = htp.tile([P, NBLK], bf16, tag="ht")
                    nc.sync.dma_start(
                        htt[:], ht[t * P:(t + 1) * P, j * NBLK:(j + 1) * NBLK]
                    )
                    nc.tensor.matmul(
                        ops[:], lhsT=y2_sb[:, t, :], rhs=htt[:],
                        start=(t == 0), stop=(t == ET - 1),
                    )
                s1 = outp.tile([F, NBLK], f32, tag="s1")
                nc.scalar.activation(out=s1[:], in_=ops[:], func=Copy)
                for c in range(NBLK // P):
                    tix = j * (NBLK // P) + c
                    t2 = ps_small.tile([P, F], f32, tag="tp")
                    nc.tensor.transpose(t2[:], s1[:, c * P:(c + 1) * P], ident[:])
                    osb = outp.tile([P, F], f32, tag="osb")
                    nc.scalar.activation(
                        out=osb[:], in_=t2[:], func=Copy,
                        scale=dv_all[:, tix:tix + 1],
                    )
                    nc.vector.tensor_tensor(
                        out=osb[:], in0=osb[:], in1=bias_sb[:], op=add
                    )
                    nc.sync.dma_start(out[tix * P:(tix + 1) * P, :], osb[:])

    nc.finalize()
    return nc


def _get_program():
    if "nc" not in _prog_cache:
        _prog_cache["nc"] = _build_program()
    return _prog_cache["nc"]


def make_in_maps(x, H, W, weight, bias):
    x = np.asarray(x, dtype=np.float32)
    H = np.asarray(H, dtype=np.float32)
    W = np.asarray(W, dtype=np.float32)
    weight = np.asarray(weight, dtype=np.float32)
    bias = np.asarray(bias, dtype=np.float32)

    H_bf = H.astype(ml_dtypes.bfloat16)
    wstr = np.ascontiguousarray(W.reshape(ET, P).T.astype(np.float32))
    biasb = np.ascontiguousarray(np.tile(bias[None, :], (P, 1)))
    wmat = np.ascontiguousarray(weight)

    in_maps = []
    for c in range(NCORES):
        hs = H_bf[c * NL:(c + 1) * NL, :]
        in_maps.append({
            "h": np.ascontiguousarray(hs),
            "ht": np.ascontiguousarray(hs.T),
            "xt": np.ascontiguousarray(x[c * NL:(c + 1) * NL, :].T),
            "wmat": wmat,
            "wstr": wstr,
            "biasb": biasb,
        })
    return in_maps


def run(x, H, W, weight, bias, trace=False, **kw):
    from concourse.bass_utils import run_bass_kernel_spmd

    nc = _get_program()
    in_maps = make_in_maps(x, H, W, weight, bias)
    res = run_bass_kernel_spmd(nc, in_maps, list(range(NCORES)), trace=trace, **kw)
    out = np.concatenate(
        [res.results[c]["out"] for c in range(NCORES)], axis=0
    ).astype(np.float32)
    return out, res


def kernel(x, H, W, weight, bias):
    out, _ = run(x, H, W, weight, bias, trace=False)
    return out


# revision 11
# speedup vs baseline: 1.2962x; 1.2962x over previous
"""HGNN conv kernel for Trainium2, 8 NeuronCores.

out = dv ⊙ (H @ (W·de ⊙ (H^T @ (dv ⊙ (x@weight))))) + bias
  dv = rowsum(H)^-1/2  [N], de = colsum(H)^-1  [E]
  N=16384, E=8192, F=64.

Sharding: H/x row-sharded over N across 8 cores (2048 rows each).
Host preps per-core bf16 H shard in both layouts (natural + transposed) —
a pure layout/precision transform; all FLOPs (matmuls, reductions,
scalings) run on device.

Device per core:
  pass 1: stream H natural [128,8192] row-tiles; DVE row-sums -> dv;
          xw = x@weight (PE); xs' = [dv*xw | 1] bf16 stationary;
          y^T[65,512-blk] += xs'^T @ H  (PSUM accum over 4-tile n-groups,
          DVE flush to f32 SBUF acc). Ones column yields colsum partials.
  AllReduce [65,8192] f32 across 8 cores.
  y2 = (W*de) * y_sum  via PE transpose + ACT scaled copy -> bf16 [e,64].
  pass 2: stream H^T [128e,512n] tiles; out^T[64,512] += y2^T @ H^T;
          PE transpose back, ACT copy scaled by dv, DVE bias add, DMA out.
"""

import numpy as np
import ml_dtypes

N, E, F = 16384, 8192, 64
NCORES = 8
NL = N // NCORES          # 2048 rows per core
P = 128
NT = NL // P              # 16 n-tiles per core
ET = E // P               # 64 e-tiles
EBLK = 512
EB = E // EBLK            # 16 e-blocks (pass 1 moving free dim)
NBLK = 512
NB = NL // NBLK           # 4 n-blocks (pass 2 moving free dim)
G = 4                     # n-tiles per PSUM accumulation group (pass 1)

_prog_cache = {}


def _build_program():
    import concourse.bass as bass
    import concourse.mybir as mybir
    import concourse.tile as tile
    from concourse import bacc
    from concourse.masks import make_identity

    f32 = mybir.dt.float32
    bf16 = mybir.dt.bfloat16
    Copy = mybir.ActivationFunctionType.Copy
    add = mybir.AluOpType.add
    mult = mybir.AluOpType.mult
    X = mybir.AxisListType.X

    nc = bacc.Bacc(
        "TRN2", target_bir_lowering=False, debug=False, num_devices=NCORES
    )
    h = nc.declare_dram_parameter("h", [NL, E], bf16, isOutput=False)
    ht = nc.declare_dram_parameter("ht", [E, NL], bf16, isOutput=False)
    xt = nc.declare_dram_parameter("xt", [F, NL], f32, isOutput=False)
    wmat = nc.declare_dram_parameter("wmat", [F, F], f32, isOutput=False)
    wstr = nc.declare_dram_parameter("wstr", [P, ET], f32, isOutput=False)
    biasb = nc.declare_dram_parameter("biasb", [P, F], f32, isOutput=False)
    out = nc.declare_dram_parameter("out", [NL, F], f32, isOutput=True)

    with tile.TileContext(nc) as tc:
        with (
            tc.tile_pool(name="hp", bufs=G + 1) as hp,           # H row tiles
            tc.tile_pool(name="xsp", bufs=G + 1) as xsp,         # xs' tiles
            tc.tile_pool(name="accp", bufs=1) as accp,           # y acc
            tc.tile_pool(name="smallp", bufs=1) as smallp,       # persistent small
            tc.tile_pool(name="rp", bufs=4) as rp,               # rowsum temps
            tc.tile_pool(name="htp", bufs=8) as htp,             # HT row tiles
            tc.tile_pool(name="outp", bufs=4) as outp,           # out staging
            tc.tile_pool(name="ps_small", bufs=2, space="PSUM") as ps_small,
            tc.tile_pool(name="ps_big", bufs=2, space="PSUM") as ps_big,
            tc.tile_pool(name="ps2", bufs=1, space="PSUM") as ps2,
            tc.tile_pool(name="dramp", bufs=1, space="DRAM") as dramp,
        ):
            # ---- persistent small tensors ----
            xt_sb = smallp.tile([F, NL], f32, tag="xt")
            nc.sync.dma_start(xt_sb[:], xt[:, :])
            wmat_sb = smallp.tile([F, F], f32, tag="wmat")
            nc.sync.dma_start(wmat_sb[:], wmat[:, :])
            wstr_sb = smallp.tile([P, ET], f32, tag="wstr")
            nc.sync.dma_start(wstr_sb[:], wstr[:, :])
            bias_sb = smallp.tile([P, F], f32, tag="bias")
            nc.sync.dma_start(bias_sb[:], biasb[:, :])
            ident = smallp.tile([F, F], f32, tag="ident")
            make_identity(nc, ident)
            dv_all = smallp.tile([P, NT], f32, tag="dv")
            y_acc = accp.tile([F + 1, E], f32, tag="yacc")

            # ---- pass 1: y^T[f,e] (+ colsum row) over n-groups ----
            for g in range(NT // G):
                group = []
                for i in range(G):
                    t = g * G + i
                    h_t = hp.tile([P, E], bf16, tag="h")
                    nc.sync.dma_start(h_t[:], h[t * P:(t + 1) * P, :])
                    # rowsum -> dv = sqrt(1/rowsum); split across DVE and ACT
                    rsum = rp.tile([P, 1], f32, tag="rsum")
                    if i % 2 == 0:
                        nc.vector.tensor_reduce(
                            out=rsum[:], in_=h_t[:], axis=X, op=add
                        )
                    else:
                        # in-place copy on ScalarE; accum_out gives the row sum
                        nc.scalar.activation(
                            out=h_t[:], in_=h_t[:], func=Copy, accum_out=rsum[:]
                        )
                    rinv = rp.tile([P, 1], f32, tag="rinv")
                    nc.vector.reciprocal(out=rinv[:], in_=rsum[:])
                    nc.scalar.sqrt(out=dv_all[:, t:t + 1], in_=rinv[:])
                    # xw = x @ weight for this tile
                    xw_ps = ps_small.tile([P, F], f32, tag="tp")
                    nc.tensor.matmul(
                        xw_ps[:], lhsT=xt_sb[:, t * P:(t + 1) * P], rhs=wmat_sb[:],
                        start=True, stop=True,
                    )
                    xs_t = xsp.tile([P, F + 1], bf16, tag="xs")
                    nc.scalar.activation(
                        out=xs_t[:, 0:F], in_=xw_ps[:], func=Copy,
                        scale=dv_all[:, t:t + 1],
                    )
                    nc.gpsimd.memset(xs_t[:, F:F + 1], 1.0)
                    group.append((xs_t, h_t))
                for b in range(EB):
                    yps = ps_big.tile([F + 1, EBLK], f32, tag="yps")
                    for i, (xs_t, h_t) in enumerate(group):
                        nc.tensor.matmul(
                            yps[:], lhsT=xs_t[:], rhs=h_t[:, b * EBLK:(b + 1) * EBLK],
                            start=(i == 0), stop=(i == G - 1),
                        )
                    dst = y_acc[:, b * EBLK:(b + 1) * EBLK]
                    if g == 0:
                        nc.vector.tensor_copy(out=dst, in_=yps[:])
                    else:
                        nc.vector.tensor_tensor(out=dst, in0=dst, in1=yps[:], op=add)

            # ---- AllReduce in 2 halves so pass 2 can start on half 0 ----
            EH = E // 2
            ETH = ET // 2
            y2_sb = smallp.tile([P, ET, F], bf16, tag="y2")
            for hf in range(2):
                b_in = dramp.tile([F + 1, EH], f32, name=f"bi{hf}")
                b_out = dramp.tile([F + 1, EH], f32, name=f"bo{hf}")
                nc.sync.dma_start(b_in[:], y_acc[:, hf * EH:(hf + 1) * EH])
                nc.gpsimd.collective_compute(
                    "AllReduce",
                    mybir.AluOpType.add,
                    ins=[b_in[:].opt()],
                    outs=[b_out[:].opt()],
                    replica_groups=[list(range(NCORES))],
                )
                # y2 = (W * de) * y_sum for this half, transposed to [e,64].
                # Reduced rows overwrite the local partial in y_acc (saves SBUF).
                nc.sync.dma_start(y_acc[0:F, hf * EH:(hf + 1) * EH], b_out[0:F, :])
                cs = smallp.tile([P, ETH], f32, name=f"cs{hf}")
                nc.sync.dma_start(
                    cs[:], b_out[F, :].rearrange("(o p) -> p o", p=P)
                )
                de_t = smallp.tile([P, ETH], f32, name=f"de{hf}")
                nc.vector.reciprocal(out=de_t[:], in_=cs[:])
                wde = smallp.tile([P, ETH], f32, name=f"wde{hf}")
                nc.vector.tensor_tensor(
                    out=wde[:], in0=de_t[:],
                    in1=wstr_sb[:, hf * ETH:(hf + 1) * ETH], op=mult,
                )
                for tt in range(ETH):
                    t = hf * ETH + tt
                    tp = ps_small.tile([P, F], f32, tag="tp")
                    nc.tensor.transpose(
                        tp[:], y_acc[0:F, t * P:(t + 1) * P], ident[:]
                    )
                    nc.scalar.activation(
                        out=y2_sb[:, t, :], in_=tp[:], func=Copy,
                        scale=wde[:, tt:tt + 1],
                    )

            # ---- pass 2: t-outer; 4 persistent PSUM banks; big HT DMAs ----
            o_tiles = [ps2.tile([F, NBLK], f32, name=f"o{j}") for j in range(NB)]
            for t in range(ET):
                htt = htp.tile([P, NL], bf16, tag="ht")
                nc.sync.dma_start(htt[:], ht[t * P:(t + 1) * P, :])
                for j in range(NB):
                    nc.tensor.matmul(
                        o_tiles[j][:], lhsT=y2_sb[:, t, :],
                        rhs=htt[:, j * NBLK:(j + 1) * NBLK],
                        start=(t == 0), stop=(t == ET - 1),
                    )
            for j in range(NB):
                s1 = outp.tile([F, NBLK], f32, tag="s1")
                nc.scalar.activation(out=s1[:], in_=o_tiles[j][:], func=Copy)
                for c in range(NBLK // P):
                    tix = j * (NBLK // P) + c
                    t2 = ps_small.tile([P, F], f32, tag="tp")
                    nc.tensor.transpose(t2[:], s1[:, c * P:(c + 1) * P], ident[:])
                    osb = outp.tile([P, F], f32, tag="osb")
                    nc.scalar.activation(
                        out=osb[:], in_=t2[:], func=Copy,
                        scale=dv_all[:, tix:tix + 1],
                    )
                    nc.vector.tensor_tensor(
                        out=osb[:], in0=osb[:], in1=bias_sb[:], op=add
                    )
                    nc.sync.dma_start(out[tix * P:(tix + 1) * P, :], osb[:])

    nc.finalize()
    return nc


def _get_program():
    if "nc" not in _prog_cache:
        _prog_cache["nc"] = _build_program()
    return _prog_cache["nc"]


def make_in_maps(x, H, W, weight, bias):
    x = np.asarray(x, dtype=np.float32)
    H = np.asarray(H, dtype=np.float32)
    W = np.asarray(W, dtype=np.float32)
    weight = np.asarray(weight, dtype=np.float32)
    bias = np.asarray(bias, dtype=np.float32)

    H_bf = H.astype(ml_dtypes.bfloat16)
    wstr = np.ascontiguousarray(W.reshape(ET, P).T.astype(np.float32))
    biasb = np.ascontiguousarray(np.tile(bias[None, :], (P, 1)))
    wmat = np.ascontiguousarray(weight)

    in_maps = []
    for c in range(NCORES):
        hs = H_bf[c * NL:(c + 1) * NL, :]
        in_maps.append({
            "h": np.ascontiguousarray(hs),
            "ht": np.ascontiguousarray(hs.T),
            "xt": np.ascontiguousarray(x[c * NL:(c + 1) * NL, :].T),
            "wmat": wmat,
            "wstr": wstr,
            "biasb": biasb,
        })
    return in_maps


def run(x, H, W, weight, bias, trace=False, **kw):
    from concourse.bass_utils import run_bass_kernel_spmd

    nc = _get_program()
    in_maps = make_in_maps(x, H, W, weight, bias)
    res = run_bass_kernel_spmd(nc, in_maps, list(range(NCORES)), trace=trace, **kw)
    out = np.concatenate(
        [res.results[c]["out"] for c in range(NCORES)], axis=0
    ).astype(np.float32)
    return out, res


def kernel(x, H, W, weight, bias):
    out, _ = run(x, H, W, weight, bias, trace=False)
    return out
